# revision 16
# baseline (speedup 1.0000x reference)
"""ContextualAttention TRN2 kernel (fp8 DoubleRow QK^T + bf16 deconv GEMM).

Full inputs -> full output. Sharding: 8 cores = 2 batches x 4 q-blocks of the
L=4096 attention-column dimension. Each core, for its 1024 columns q:

  S[p, q]   = sum_f wn16[f, p] * prq8[f, q]     (fp8e4 DoubleRow GEMM, K padded
                                                 to 1280 = 5 pairs of 128-row
                                                 planes; two extra contraction
                                                 rows carry the per-column
                                                 softmax shift -16*d_q so no
                                                 separate bias matmul is needed)
  E[p, q]   = exp(0.625*S + mrow_p)             (= exp(10*(S/16 - d_q) + mask),
                                                 scalar engine, fp8e4 output)
  cs[q]     = sum_p E[p, q]                     (DoubleRow GEMM with ones lhsT)
  col[q,ck] = sum_p E[p, q] * xu[p, ck]         (fp8 E stationary x bf16 xu
                                                 moving; raw, un-normalized)

The softmax normalization (col[q,:] /= cs[q]) and the col2im overlap-add run
on the host, as does the unfold/normalize prep.  The softmax is exact despite
fp8: A[:,q] = E/cs uses the same quantized E in numerator and denominator, so
quantization of the dominant diagonal cancels; off-diagonal terms underflow to
zero in fp32 exactly as they do in the reference.
"""
import numpy as np
import ml_dtypes

import concourse.bass as bass
import concourse.bacc as bacc
import concourse.mybir as mybir
from concourse import tile
from concourse.bass_utils import run_bass_kernel_spmd

F32 = mybir.dt.float32
BF16 = mybir.dt.bfloat16
F8 = mybir.dt.float8e4
AFT = mybir.ActivationFunctionType
DR = mybir.MatmulPerfMode.DoubleRow
E4M3 = ml_dtypes.float8_e4m3
BF16NP = ml_dtypes.bfloat16

B, C, H, W = 2, 128, 128, 128
RATE, BS = 2, 3                # attention rate, block size
Hr, Wr = H // RATE, W // RATE  # 64, 64
L = Hr * Wr                    # 4096
F = C * BS * BS                # 1152 true contraction dim
FP = 1280                      # padded contraction: 5 DoubleRow pairs of 256
NKP = FP // 256                # 5 k-pairs
CK = C * 16                    # 2048 deconv output cols (kappa*128 + c)
QPC = L // 4                   # 1024 q columns per core
NPT = L // 128                 # 32 p tiles
NQT = QPC // 128               # 8 q tiles
NCH = CK // 512                # 4 ck chunks
EPS = 1e-4
SCALE = 10.0
WSC = 16.0                     # host pre-scale on wn for fp8 range
N_CORES = 8

_CACHE = {}


def _build_nc():
    nc = bacc.Bacc(None)
    # wn/prq pre-transposed on host to partition-major layout so every DMA
    # line is contiguous (128B descriptors otherwise throttle the queue)
    wn_d = nc.declare_dram_parameter("wn", [L, FP], F8, isOutput=False)
    prq_d = nc.declare_dram_parameter("prq", [128, NKP * 2 * QPC], F8,
                                      isOutput=False)
    xu_d = nc.declare_dram_parameter("xu", [L, CK], BF16, isOutput=False)
    mrow_d = nc.declare_dram_parameter("mrow", [128, NPT], F32, isOutput=False)
    ones_d = nc.declare_dram_parameter("ones2", [128, 32], F8, isOutput=False)
    col_d = nc.declare_dram_parameter("col", [QPC, CK], F32, isOutput=True)
    cs_d = nc.declare_dram_parameter("cs", [1, QPC], F32, isOutput=True)

    with tile.TileContext(nc) as tc:
        with (
            tc.tile_pool(name="const", bufs=1) as cpool,
            tc.tile_pool(name="rhs", bufs=1) as rhspool,
            tc.tile_pool(name="lhs", bufs=6) as lhspool,
            tc.tile_pool(name="afull", bufs=1) as apool,
            tc.tile_pool(name="xus", bufs=1) as xupool,
            tc.tile_pool(name="outs", bufs=4) as opool,
            tc.tile_pool(name="csb", bufs=1) as cspool,
            tc.tile_pool(name="ps", bufs=8, space="PSUM") as pspool,
        ):
            # ---- resident loads -------------------------------------------
            rhs_sb = rhspool.tile([128, NKP * 2, QPC], F8)   # 10 KB/part
            for kp in range(NKP):  # per k-pair so the first matmul starts early
                nc.sync.dma_start(
                    rhs_sb[:, 2 * kp:2 * kp + 2, :]
                    .rearrange("p k q -> p (k q)"),
                    prq_d[:, 2 * kp * QPC:(2 * kp + 2) * QPC])
            m_sb = cpool.tile([128, NPT], F32)
            nc.sync.dma_start(m_sb[:], mrow_d[:])
            ones_sb = cpool.tile([128, 2, 16], F8)
            nc.sync.dma_start(
                ones_sb[:], ones_d[:].rearrange("p (t i) -> p t i", t=2))
            a_full = apool.tile([128, NPT, QPC], F8)         # 32 KB/part

            # xu resident, one tile per p-tile (prefetch streams during A on
            # queues not used by the phase-A lhs loads; the last third rides
            # the sync queue behind the small wn tiles)
            xu_t = [xupool.tile([128, CK], BF16, name=f"xu{pt}")
                    for pt in range(NPT)]
            xq = [nc.scalar, nc.gpsimd]
            for pt in range(NPT):
                xq[pt % 2].dma_start(
                    xu_t[pt][:], xu_d[pt * 128:(pt + 1) * 128, :])

            # ---- phase A: S = wn16^T prq8 (DR fp8), E = exp -> a_full -----
            for pt in range(NPT):
                lhs = lhspool.tile([128, NKP * 2, 128], F8)
                nc.sync.dma_start(
                    lhs[:].rearrange("p k j -> p (k j)"),
                    wn_d[pt * 128:(pt + 1) * 128, :])
                ps0 = pspool.tile([128, 512], F32, tag="ps")
                ps1 = pspool.tile([128, 512], F32, tag="ps")
                pss = (ps0, ps1)
                for kp in range(NKP):      # stationary reused for both chunks
                    for qc in range(2):
                        nc.tensor.matmul(
                            pss[qc][:],
                            lhs[:, 2 * kp:2 * kp + 2, :],
                            rhs_sb[:, 2 * kp:2 * kp + 2,
                                   qc * 512:(qc + 1) * 512],
                            start=(kp == 0), stop=(kp == NKP - 1),
                            perf_mode=DR)
                for qc in range(2):
                    nc.scalar.activation(
                        a_full[:, pt:pt + 1, qc * 512:(qc + 1) * 512]
                        .rearrange("p a n -> p (a n)"),
                        pss[qc][:], AFT.Exp,
                        bias=m_sb[:, pt:pt + 1], scale=SCALE / WSC)

            # ---- phase B: cs[q] = sum_p E (DR, ones stationary) -----------
            cs_sb = cspool.tile([1, QPC], F32)
            for qc in range(2):
                cs_ps = pspool.tile([1, 512], F32, tag="ps", name=f"cs{qc}")
                for t in range(NPT // 2):
                    nc.tensor.matmul(
                        cs_ps[:], ones_sb[:, :, 0:1],
                        a_full[:, 2 * t:2 * t + 2,
                               qc * 512:(qc + 1) * 512],
                        start=(t == 0), stop=(t == NPT // 2 - 1),
                        perf_mode=DR)
                nc.vector.tensor_copy(cs_sb[:, qc * 512:(qc + 1) * 512],
                                      cs_ps[:])
            nc.sync.dma_start(cs_d[:], cs_sb[:])

            # ---- phase C: col[q, ck] = sum_p E[p,q] xu[p,ck] (raw) --------
            for ch in range(NCH):
                ps_c = [pspool.tile([128, 512], F32, tag="ps",
                                    name=f"psc{ch}_{i}")
                        for i in range(NQT)]
                for pt in range(NPT):
                    for qt in range(NQT):
                        nc.tensor.matmul(
                            ps_c[qt][:],
                            a_full[:, pt:pt + 1,
                                   qt * 128:(qt + 1) * 128]
                            .rearrange("p a n -> p (a n)"),
                            xu_t[pt][:, ch * 512:(ch + 1) * 512],
                            start=(pt == 0), stop=(pt == NPT - 1))
                for qt in range(NQT):
                    ot = opool.tile([128, 512], F32)
                    if qt % 2:
                        nc.vector.tensor_copy(ot[:], ps_c[qt][:])
                    else:
                        nc.scalar.activation(ot[:], ps_c[qt][:], AFT.Copy)
                    (nc.sync if qt % 2 else nc.gpsimd).dma_start(
                        col_d[qt * 128:(qt + 1) * 128,
                              ch * 512:(ch + 1) * 512], ot[:])
    nc.compile()
    return nc


def _host_prep(x, mask):
    """Per-batch GEMM-ready operands (kappa-major feature layout)."""
    out = []
    for b in range(B):
        xr = x[b, :, ::RATE, ::RATE]
        xrp = np.pad(xr, ((0, 0), (1, 1), (1, 1)))
        pr = np.empty((9, C, L), np.float32)
        for di in range(3):
            for dj in range(3):
                pr[di * 3 + dj] = xrp[:, di:di + Hr, dj:dj + Wr].reshape(C, L)
        pr = pr.reshape(F, L)
        denom = np.sqrt((pr * pr).sum(0, dtype=np.float64).astype(np.float32)
                        + np.float32(F * EPS))

        mr = mask[b, :, ::RATE, ::RATE]
        mrp = np.pad(mr, ((0, 0), (1, 1), (1, 1)))
        msum = np.zeros((1, L), np.float32)
        for di in range(3):
            for dj in range(3):
                msum += mrp[:, di:di + Hr, dj:dj + Wr].reshape(1, L)
        mfilt = (msum[0] == 0.0).astype(np.float32)

        wn = (pr / denom[None, :]) * mfilt[None, :]
        wn8 = np.zeros((FP, L), E4M3)
        wn8[:F] = (WSC * wn).astype(E4M3)
        wn8[F:F + 3] = np.float32(WSC).astype(E4M3)  # bias rows: weight = 16

        # three shift rows summing to ~-d_q (the third absorbs the e4m3
        # rounding of the second, keeping exp() within fp8 range)
        prq8 = np.zeros((FP, L), E4M3)
        prq8[:F] = pr.astype(E4M3)
        prq8[F] = np.float32(-32.0).astype(E4M3)
        r2 = (32.0 - denom).astype(E4M3)
        prq8[F + 1] = r2
        resid = denom - 32.0 + r2.astype(np.float32)
        prq8[F + 2] = (-resid).astype(E4M3)

        xp = np.pad(x[b], ((0, 0), (1, 1), (1, 1)))
        xu = np.empty((L, 16, C), np.float32)
        for i in range(4):
            for j in range(4):
                blk = xp[:, i:i + 2 * Hr:2, j:j + 2 * Wr:2]
                xu[:, i * 4 + j, :] = blk.reshape(C, L).T
        # partition-major relayouts: wnt[pt*128+fi, k*128+j] = wn8[k*128+fi,
        # pt*128+j] so each on-device DMA line is contiguous
        wnt = np.ascontiguousarray(
            wn8.reshape(NKP * 2, 128, NPT, 128)
            .transpose(2, 1, 0, 3).reshape(L, FP))
        out.append((wnt, prq8,
                    ((mfilt - 1.0) * 1e4 - 0.5).reshape(NPT, 128).T,
                    np.ascontiguousarray(xu.reshape(L, CK).astype(BF16NP))))
    return out


def _col2im(col):
    """col [L, CK] -> [C, H, W] overlap-add, /4."""
    canvas = np.zeros((C, H + 2, W + 2), np.float32)
    blk = col.reshape(Hr, Wr, 16, C)
    for i in range(4):
        for j in range(4):
            canvas[:, i:i + 2 * Hr:2, j:j + 2 * Wr:2] += \
                blk[:, :, i * 4 + j, :].transpose(2, 0, 1)
    return canvas[:, 1:1 + H, 1:1 + W] / 4.0


def kernel(x, mask):
    x = np.asarray(x, np.float32)
    mask = np.asarray(mask, np.float32)
    if "nc" not in _CACHE:
        _CACHE["nc"] = _build_nc()
    nc = _CACHE["nc"]

    prep = _host_prep(x, mask)
    ones2 = np.ones((128, 32), np.float32).astype(E4M3)
    in_maps = []
    for core in range(N_CORES):
        b, g = divmod(core, 4)
        wnt, prq8, mrow, xu = prep[b]
        q0 = g * QPC
        prqc = np.ascontiguousarray(
            prq8[:, q0:q0 + QPC].reshape(NKP * 2, 128, QPC)
            .transpose(1, 0, 2).reshape(128, NKP * 2 * QPC))
        in_maps.append({
            "wn": wnt,
            "prq": prqc,
            "xu": xu,
            "mrow": np.ascontiguousarray(mrow),
            "ones2": ones2,
        })

    _CACHE["in_maps"] = in_maps
    res = run_bass_kernel_spmd(nc, in_maps, list(range(N_CORES)))

    out = np.empty((B, C, H, W), np.float32)
    for b in range(B):
        parts = []
        for g in range(4):
            r = res.results[b * 4 + g]
            cs = r["cs"][0].astype(np.float32)
            cs = np.where(cs == 0.0, 1.0, cs)
            parts.append(r["col"] / cs[:, None])
        out[b] = _col2im(np.concatenate(parts, axis=0))
    return out
